# revision 38
# baseline (speedup 1.0000x reference)
"""CDConvBlock Trainium2 kernel (8-core SPMD, data-parallel over batch).

Math (per image, x: [C, H*W] channel-major):
    offset = tanh(w_off @ x + b_off)                      [2, HW]
    gx = clip(gx_base + offset[0], -1, 1), gy likewise
    A' = (w_W @ w_pc) * w_dw[None, :]                     [C, C]
    out = bilinear_zeros(A' @ x, gx, gy) + b_W + x
The two 1x1 convs and the depthwise scale commute with the per-channel
bilinear gather, so they all fold into the single matrix A' applied
BEFORE sampling.

Device pipeline (per core = one image), software-pipelined so phase D
starts while phase B is still streaming:
  B-pass 1 (y path): stream x (bf16, host-cast).  Per 128-pixel chunk
     an x-stationary matmul against rhs = [A'.T | w_off.T] yields a
     PIXEL-major psum [128px, 258] = [y row | dx_pre | dy_pre]; y rows
     are cast to fp8 and written to DRAM twice in a PAIR layout (token
     (r, x) holds rows r AND r+1, 512 ch) so one gather element later
     covers all 4 bilinear corners.  Per 16-row batch, offset math on
     small [128, 16] tiles (pipelined one batch behind the matmuls)
     produces per-batch weight tiles wgt4_t[x, k, y] and gather-index
     tiles ibT_t (selector matmuls fold the pixel-major index field
     into the DMA-gather 16-partition-wrapped layout).
  B-pass 2 (residual path, drains on PE/ACT while phase D runs):
     re-stream x, identity matmuls transpose it to pixel-major, a
     one-hot matmul injects b_W, and the x+b_W seed is written
     straight to the output tensor.
  D (per 8-row chunk): ONE dma_gather fetches 1KB elements (2 adjacent
     pair tokens = 4 corners x 256 ch fp8); the gather source AP is
     windowed to rows <= 8k+72 so each chunk only waits on the y
     writes it can actually read.  DVE combine: one broadcast multiply
     M = G * wgt4[..., bcast c] and a 3-add tree t = (M0+M1)+(M2+M3);
     a gpsimd DMA accumulates t onto the pre-written seed
     (accum_op=add), two chunks per write.
Host only shards/reshapes (batch split, bf16 cast, weight transposes,
constant tables, final layout transpose).
"""

import numpy as np

import concourse.bass as bass
import concourse.bacc as bacc
import concourse.mybir as mybir
from concourse.bass_types import AP
from concourse.tile import TileContext
from concourse.tile_rust import add_dep_helper
from concourse import bass_utils
from concourse import library_config

F32 = mybir.dt.float32
BF16 = mybir.dt.bfloat16
I16 = mybir.dt.int16
FP8 = mybir.dt.float8e4
NP_BF16 = mybir.dt.np(BF16)
OP = mybir.AluOpType
AF = mybir.ActivationFunctionType

B, C, H, W = 8, 256, 128, 128
HW = H * W
N_CORES = 8

XB = 16     # image rows per x-stream batch
GC = 8      # image rows per gather chunk

_CACHE = {}


def _build(reps=1):
    nc = bacc.Bacc(
        "TRN2", target_bir_lowering=False, debug=False, num_devices=N_CORES
    )
    x_d = nc.dram_tensor("x", [C, HW], BF16, kind="ExternalInput")
    wpc_d = nc.dram_tensor("w_pc", [C, C], F32, kind="ExternalInput")
    wwt_d = nc.dram_tensor("w_W_T", [C, C], F32, kind="ExternalInput")
    wdw_d = nc.dram_tensor("w_dw_p", [128, 2], F32, kind="ExternalInput")
    wofft_d = nc.dram_tensor("w_off_T", [C, 2], F32, kind="ExternalInput")
    boff_d = nc.dram_tensor("b_off_b", [1, 2], F32, kind="ExternalInput")
    bw0_d = nc.dram_tensor("bw0", [128, C], F32, kind="ExternalInput")
    e0_d = nc.dram_tensor("e0", [128, 128], F32, kind="ExternalInput")
    gxb_d = nc.dram_tensor("gxb", [W, 1], F32, kind="ExternalInput")
    gyb_d = nc.dram_tensor("gyb", [1, H], F32, kind="ExternalInput")
    id0_d = nc.dram_tensor("id0", [128, C], F32, kind="ExternalInput")
    id1_d = nc.dram_tensor("id1", [128, C], F32, kind="ExternalInput")
    sel_d = nc.dram_tensor("sel", [128, 8 * 128], F32, kind="ExternalInput")
    out_d = nc.dram_tensor("out_pm", [HW, C], BF16, kind="ExternalOutput")

    with TileContext(nc) as tc:
        for _ in range(reps):
            _kernel_body(
                nc, tc, x_d, wpc_d, wwt_d, wdw_d, wofft_d, boff_d, bw0_d,
                e0_d, gxb_d, gyb_d, id0_d, id1_d, sel_d, out_d,
            )
    nc.finalize()
    return nc


def _kernel_body(
    nc, tc, x_d, wpc_d, wwt_d, wdw_d, wofft_d, boff_d, bw0_d,
    e0_d, gxb_d, gyb_d, id0_d, id1_d, sel_d, out_d,
):
    v = nc.vector
    g = nc.gpsimd
    s = nc.scalar

    with (
        tc.tile_pool(name="persist", bufs=1) as pp,
        tc.tile_pool(name="wload", bufs=1) as wp,
        tc.tile_pool(name="scr", bufs=2) as scr,
        tc.tile_pool(name="xs", bufs=2) as xp,
        tc.tile_pool(name="yst", bufs=2) as yp,
        tc.tile_pool(name="xst", bufs=2) as xsp,
        tc.tile_pool(name="gat", bufs=3) as gp,
        tc.tile_pool(name="mul", bufs=2) as mp,
        tc.tile_pool(name="fld", bufs=2) as fp_,
        tc.tile_pool(name="psA", bufs=4, space="PSUM") as psa,
        tc.tile_pool(name="dram", bufs=1, space="DRAM") as dp,
    ):
        # gpsimd ucode library for DMAGatherAnt
        lib_load = g.load_library(library_config.mlp)

        # ---------------- Phase A: weight prep ----------------
        wpc_sb = [wp.tile([128, C], F32, tag=f"wpc{m}", name=f"wpc{m}") for m in range(2)]
        wwt_sb = [wp.tile([128, C], F32, tag=f"wwt{m}", name=f"wwt{m}") for m in range(2)]
        wdw_sb = wp.tile([128, 2], F32, tag="wdw", name="wdw")
        wofft_sb = [wp.tile([128, 2], F32, tag=f"woft{m}", name=f"woft{m}") for m in range(2)]
        for m in range(2):
            nc.sync.dma_start(wpc_sb[m][:], wpc_d.ap()[m * 128:(m + 1) * 128, :])
            nc.sync.dma_start(wwt_sb[m][:], wwt_d.ap()[m * 128:(m + 1) * 128, :])
            nc.sync.dma_start(
                wofft_sb[m][:], wofft_d.ap()[m * 128:(m + 1) * 128, :]
            )
        nc.sync.dma_start(wdw_sb[:], wdw_d.ap())

        boff_sb = pp.tile([128, 2], F32, tag="boff", name="boff")
        gxb_sb = pp.tile([128, 1], F32, tag="gxb", name="gxb")
        gyb_sb = pp.tile([128, H], F32, tag="gyb", name="gyb")
        nc.sync.dma_start(boff_sb[:], boff_d.ap().to_broadcast((128, 2)))
        nc.sync.dma_start(gxb_sb[:], gxb_d.ap())
        nc.sync.dma_start(gyb_sb[:], gyb_d.ap().to_broadcast((128, H)))

        id_sb = [pp.tile([128, C], BF16, tag=f"id{m}", name=f"id{m}") for m in range(2)]
        g.dma_start(id_sb[0][:], id0_d.ap())  # f32 -> bf16 cast
        g.dma_start(id_sb[1][:], id1_d.ap())
        e0_sb = pp.tile([128, 128], BF16, tag="e0", name="e0")
        bw0_sb = pp.tile([128, C], BF16, tag="bw0", name="bw0")
        g.dma_start(e0_sb[:], e0_d.ap())
        g.dma_start(bw0_sb[:], bw0_d.ap())
        sel_sb = pp.tile([128, 8 * 128], F32, tag="sel", name="sel")
        nc.sync.dma_start(sel_sb[:], sel_d.ap())

        # B matrices: B[cb] = [A'.T | w_off.T] rows cb*128..cb*128+128
        B_sb = [pp.tile([128, C + 2], BF16, tag=f"B{cb}", name=f"B{cb}") for cb in range(2)]
        for cb in range(2):
            ps = psa.tile([128, 2, 512], F32, tag="psA", name="psA")
            for m in range(2):
                nc.tensor.matmul(
                    ps[:, 0, 0:C],
                    wpc_sb[m][:, cb * 128:(cb + 1) * 128],
                    wwt_sb[m][:],
                    start=(m == 0),
                    stop=(m == 1),
                )
            v.tensor_scalar(
                B_sb[cb][:, 0:C], ps[:, 0, 0:C], wdw_sb[:, cb:cb + 1], None,
                OP.mult,
            )
            v.tensor_copy(B_sb[cb][:, C:C + 2], wofft_sb[cb][:])

        # per-batch state tiles (small, one set per x-stream batch so
        # phase-D consumers depend only on their batch, not all of phase B)
        n_xbat = H // XB
        offs_t = [pp.tile([128, 2, XB], F32, tag=f"offs{t}", name=f"offs{t}")
                  for t in range(n_xbat)]
        wgt4_t = [pp.tile([128, 4, XB], BF16, tag=f"wgt4_{t}", name=f"wgt4_{t}")
                  for t in range(n_xbat)]
        ibT_t = [pp.tile([128, XB * 8], I16, tag=f"ibT{t}", name=f"ibT{t}")
                 for t in range(n_xbat)]

        # PAIR-token layout: token j = y*128+x holds fp8 y rows y and y+1
        # (512 ch).  8.4 MB.
        y_dram = dp.tile([HW * 2 * C], FP8, tag="ydram", name="ydram")

        # Pair row 127's bottom half (y row 128) is never gathered but sits
        # inside the gather source window — zero it so validation passes.
        zpad = pp.tile([128, C], FP8, tag="zpad", name="zpad")
        v.memset(zpad[:], 0.0)

        # ---------------- Phase C helpers (per batch) ----------------
        def axis_pipeline(o, base_is_tensor, t, eng=v):
            """offs_t[t][:, o, :] -> (b (f32 col base), wA, wB) for one axis."""
            ys, ye = t * XB, (t + 1) * XB
            d = scr.tile([128, XB], F32, tag="sc_d", name="sc_d")
            s.activation(d, offs_t[t][:, o, :], AF.Tanh, bias=boff_sb[:, o:o + 1])
            gg = scr.tile([128, XB], F32, tag="sc_g", name="sc_g")
            if base_is_tensor:
                eng.scalar_tensor_tensor(gg, d, 1.0, gyb_sb[:, ys:ye], OP.mult, OP.add)
            else:
                eng.tensor_scalar(gg, d, gxb_sb[:, 0:1], None, OP.add)
            # u = ix + 384 lives in the f32 binade [256, 512) where ulp is
            # 2^-15, so clearing the low 15 mantissa bits IS floor(u) for
            # the whole range ix in [-0.5, 127.5].  All exact.
            ix = scr.tile([128, XB], F32, tag="sc_ix", name="sc_ix")
            eng.tensor_scalar(ix, gg, -1.0, 1.0, OP.max, OP.min)
            eng.tensor_scalar(ix, ix, 64.0, 63.5 + 384.0, OP.mult, OP.add)
            ufl = scr.tile([128, XB], F32, tag="sc_ufl", name="sc_ufl")
            eng.tensor_scalar(
                ufl[:].bitcast(mybir.dt.uint32), ix[:].bitcast(mybir.dt.uint32),
                0xFFFF8000, None, OP.bitwise_and,
            )
            fx = scr.tile([128, XB], F32, tag="sc_fx", name="sc_fx")
            eng.tensor_tensor(fx, ix, ufl, OP.subtract)
            mneg = scr.tile([128, XB], F32, tag="sc_mneg", name="sc_mneg")
            eng.tensor_scalar(mneg, ufl, 383.5, None, OP.is_lt)
            mhi = scr.tile([128, XB], F32, tag="sc_mhi", name="sc_mhi")
            eng.tensor_scalar(mhi, ufl, 510.5, None, OP.is_gt)
            # clamped base, still in u-domain (bx + 384)
            bcol = scr.tile([128, XB], F32, tag="sc_b", name="sc_b")
            eng.tensor_scalar(bcol, ufl, 384.0, 510.0, OP.max, OP.min)
            # common = 1 - mneg - mhi;  g1 = 1 - fx
            cm = scr.tile([128, XB], F32, tag="sc_cm", name="sc_cm")
            eng.tensor_tensor(cm, mneg, mhi, OP.add)
            eng.tensor_scalar(cm, cm, -1.0, 1.0, OP.mult, OP.add)
            g1 = scr.tile([128, XB], F32, tag="sc_g1", name="sc_g1")
            eng.tensor_scalar(g1, fx, -1.0, 1.0, OP.mult, OP.add)
            # wA = g1*common + fx*mneg ; wB = fx*common + g1*mhi
            wA = scr.tile([128, XB], F32, tag="sc_wA", name="sc_wA")
            t1 = scr.tile([128, XB], F32, tag="sc_t1", name="sc_t1")
            eng.tensor_tensor(wA, g1, cm, OP.mult)
            eng.tensor_tensor(t1, fx, mneg, OP.mult)
            eng.tensor_tensor(wA, wA, t1, OP.add)
            eng.tensor_scalar(wA, wA, 0.25, None, OP.mult)
            wB_ = scr.tile([128, XB], F32, tag="sc_wB", name="sc_wB")
            eng.tensor_tensor(wB_, fx, cm, OP.mult)
            eng.tensor_tensor(t1, g1, mhi, OP.mult)
            eng.tensor_tensor(wB_, wB_, t1, OP.add)
            eng.tensor_scalar(wB_, wB_, 0.25, None, OP.mult)
            return bcol, wA, wB_

        idf_t = {}

        def build_weights(t):
            """DVE/ACT part: corner weights + f32 token index for batch t."""
            bx, wL, wR = axis_pipeline(0, False, t)
            by, wT, wB_ = axis_pipeline(1, True, t)
            # gather corner order within a 1KB element: [v00, v10, v01, v11]
            v.tensor_tensor(wgt4_t[t][:, 0, :], wT, wL, OP.mult)
            v.tensor_tensor(wgt4_t[t][:, 1, :], wB_, wL, OP.mult)
            v.tensor_tensor(wgt4_t[t][:, 2, :], wT, wR, OP.mult)
            v.tensor_tensor(wgt4_t[t][:, 3, :], wB_, wR, OP.mult)
            # pair-token index: idx = by*128 + bx.  bx/by are still in the
            # u-domain (+384 each): subtract 384*129.
            idf = pp.tile([128, XB], F32, tag=f"idf{t}", name=f"idf{t}")
            v.scalar_tensor_tensor(idf, by, 128.0, bx, OP.mult, OP.add)
            v.tensor_scalar(idf, idf, -384.0 * 129.0, None, OP.add)
            idf_t[t] = idf

        def build_indices(t):
            """PE part: fold idf into the gather index layout
            ibT_t[t][p, y*8+g] = idf[g*16 + p%16, y] via 8 selector matmuls
            into disjoint psum columns + one strided cast copy."""
            psS = psa.tile([128, 2, 512], F32, tag="psA", name="psS")
            for q in range(8):
                nc.tensor.matmul(
                    psS[:, 0, q * XB:(q + 1) * XB],
                    sel_sb[:, q * 128:(q + 1) * 128],
                    idf_t[t][:],
                    start=True,
                    stop=True,
                    skip_group_check=True,
                )
            src = (
                psS[:, 0, 0:XB * 8]
                .rearrange("p (q y) -> p q y", y=XB)
                .transpose([0, 2, 1])
            )
            v.tensor_copy(
                ibT_t[t][:].rearrange("p (y q) -> p y q", q=8), src
            )

        # ---------------- Phase B: matmuls, y tokens, residual seed ----------------
        y_writes = []       # pair-token writes (gather RAW deps)
        seed_writes = []    # out_pm seed writes (accum RAW deps)
        y2full = y_dram[:].rearrange("(j c) -> j c", c=2 * C)
        zv = y2full[(H - 1) * 128:HW, C:2 * C].rearrange("(r x) c -> x r c", x=128)
        zpad_write = nc.sync.dma_start(zv, zpad[:].unsqueeze(1))
        # ---- pass 1: y-token path only, so gathers can start early ----
        for t in range(n_xbat):
            x0 = xp.tile([128, XB * 128], BF16, tag="x0", name="x0")
            x1 = xp.tile([128, XB * 128], BF16, tag="x1", name="x1")
            cols = slice(t * XB * 128, (t + 1) * XB * 128)
            nc.sync.dma_start(x0[:], x_d.ap()[0:128, cols])
            nc.sync.dma_start(x1[:], x_d.ap()[128:256, cols])
            # weights/index math for the PREVIOUS batch: its ACT/DVE ops run
            # while this batch's matmuls stream, and the PE fold lands after
            # them, so no engine waits on a cross-engine chain.
            if t > 0:
                build_weights(t - 1)
            yst = yp.tile([128, XB, C], FP8, tag="yst", name="yst")
            for hb in range(XB // 2):
                psA = psa.tile([128, 2, 512], F32, tag="psA", name="psA")
                for r in range(2):
                    cc = (hb * 2 + r) * 128
                    lhs0 = x0[:, cc:cc + 128]
                    lhs1 = x1[:, cc:cc + 128]
                    nc.tensor.matmul(
                        psA[:, r, 0:C + 2], lhs0, B_sb[0][:], start=True, stop=False
                    )
                    nc.tensor.matmul(
                        psA[:, r, 0:C + 2], lhs1, B_sb[1][:], start=False, stop=True
                    )
                rr = hb * 2
                # y tokens -> fp8 staging (ACT)
                s.copy(yst[:, rr:rr + 2, :], psA[:, :, 0:C])
                # offsets: [x, r, o] -> offs_t[t][x, o, rr+r]
                v.tensor_copy(
                    offs_t[t][:, :, rr:rr + 2].transpose([0, 2, 1]),
                    psA[:, :, C:C + 2],
                )
            if t > 0:
                build_indices(t - 1)
            rows = slice(t * XB * 128, (t + 1) * XB * 128)
            # write 1: top half of pair tokens (t*XB .. t*XB+XB-1)
            yv1 = y2full[rows, 0:C].rearrange("(r x) c -> x r c", x=128)
            y_writes.append(nc.sync.dma_start(yv1, yst[:]))
            # write 2: bottom half of pair tokens (t*XB-1 .. t*XB+XB-2)
            if t == 0:
                yv2 = y2full[0:(XB - 1) * 128, C:2 * C].rearrange(
                    "(r x) c -> x r c", x=128
                )
                y_writes.append(nc.sync.dma_start(yv2, yst[:, 1:XB, :]))
            else:
                yv2 = y2full[(t * XB - 1) * 128:(t * XB - 1 + XB) * 128, C:2 * C].rearrange(
                    "(r x) c -> x r c", x=128
                )
                y_writes.append(nc.sync.dma_start(yv2, yst[:]))
        build_weights(n_xbat - 1)
        build_indices(n_xbat - 1)

        # ---- pass 2: residual transpose path; PE/ACT drain it during
        # phase D (x is re-streamed, DMA has headroom there) ----
        for t in range(n_xbat):
            x0 = xp.tile([128, XB * 128], BF16, tag="x0", name="x0")
            x1 = xp.tile([128, XB * 128], BF16, tag="x1", name="x1")
            cols = slice(t * XB * 128, (t + 1) * XB * 128)
            nc.sync.dma_start(x0[:], x_d.ap()[0:128, cols])
            nc.sync.dma_start(x1[:], x_d.ap()[128:256, cols])
            xst = xsp.tile([128, XB, C], BF16, tag="xst", name="xst")
            for hb in range(XB // 2):
                psB = psa.tile([128, 2, 512], F32, tag="psA", name="psB")
                for r in range(2):
                    cc = (hb * 2 + r) * 128
                    nc.tensor.matmul(
                        psB[:, r, 0:C], x0[:, cc:cc + 128], id_sb[0][:],
                        start=True, stop=False,
                    )
                    nc.tensor.matmul(
                        psB[:, r, 0:C], x1[:, cc:cc + 128], id_sb[1][:],
                        start=False, stop=False,
                    )
                    # inject b_W: psB[m, :] += bw0[0, :]  (E0 one-hot)
                    nc.tensor.matmul(
                        psB[:, r, 0:C], e0_sb[:], bw0_sb[:], start=False, stop=True
                    )
                rr = hb * 2
                # residual seed x + b_W -> bf16 staging (ACT)
                s.copy(xst[:, rr:rr + 2, :], psB[:, :, 0:C])
            rows = slice(t * XB * 128, (t + 1) * XB * 128)
            ov = out_d.ap()[rows, :].rearrange("(r x) c -> x r c", x=128)
            seed_writes.append(nc.sync.dma_start(ov, xst[:]))

        # ---------------- Phase D: gather + bilinear combine ----------------
        n_chunk = H // GC
        nidx = GC * 128
        for k in range(n_chunk):
            gD = gp.tile([128, GC, 4 * C], FP8, tag="gD", name="gD")
            tb = k // 2
            half = (k % 2) * (XB // 2) * 8
            bmax = min(n_xbat - 1, (GC * k + GC - 1 + 65) // XB)
            # gather source window: pair rows < wrows (limits RAW deps).
            # Exact bound: chunk k touches pair rows <= 8k+71, whose bottom
            # halves come from y rows <= 8k+72.
            wrows = min(H, GC * k + GC + 65)
            y2d = AP(
                y_dram[:].tensor, y_dram[:].offset,
                [[2 * C, wrows * 128 - 1], [1, 4 * C]],
            )
            gi = g.dma_gather(
                gD[:], y2d, ibT_t[tb][:, half:half + nidx // 16], nidx, nidx,
                elem_size=4 * C, elem_step=2 * C,
            )
            add_dep_helper(gi.ins, lib_load.ins, reason="gather needs mlp lib")
            if wrows >= H:
                add_dep_helper(gi.ins, zpad_write.ins, reason="gather window pad")
            for wr in y_writes[:2 * (bmax + 1)]:
                add_dep_helper(gi.ins, wr.ins, reason="gather reads y_dram")
            # combine: M = G * w  (weight bcast over channels), then the
            # 3-add fold tree; the out DMA accumulates onto the seed.
            M = mp.tile([128, GC, 4, C], BF16, tag="M", name="M")
            wv = (
                wgt4_t[tb][:, :, (k % 2) * GC:(k % 2) * GC + GC]
                .transpose([0, 2, 1])
                .unsqueeze(3)
                .broadcast_to((128, GC, 4, C))
            )
            gv = gD[:].rearrange("p y (k c) -> p y k c", k=4)
            v.tensor_tensor(M[:], gv, wv, OP.mult)
            A_ = fp_.tile([128, GC, C], BF16, tag="fa", name="fa")
            B2 = fp_.tile([128, GC, C], BF16, tag="fb", name="fb")
            if k % 2 == 0:
                T2 = fp_.tile([128, 2, GC, C], BF16, tag="ft", name="ft")
            # late chunks: gathers have drained, so GpSimd takes two of
            # the three fold-adds off the DVE critical path
            fold_eng = g if k >= 10 else v
            v.tensor_tensor(A_[:], M[:, :, 0, :], M[:, :, 1, :], OP.add)
            fold_eng.tensor_tensor(B2[:], M[:, :, 2, :], M[:, :, 3, :], OP.add)
            fold_eng.tensor_tensor(T2[:, k % 2], A_[:], B2[:], OP.add)
            if k % 2 == 1:
                ov = out_d.ap()[(k - 1) * nidx:(k + 1) * nidx, :].rearrange(
                    "(r x) c -> x r c", x=128
                )
                ow = g.dma_start(ov, T2[:], accum_op=OP.add)
                add_dep_helper(
                    ow.ins, seed_writes[(GC * k) // XB].ins,
                    reason="accum needs seed written first",
                )


def _sel_const():
    sel = np.zeros((128, 8, 128), dtype=np.float32)
    for gq in range(8):
        for p in range(128):
            sel[gq * 16 + p % 16, gq, p] = 1.0
    return sel.reshape(128, 8 * 128)


def _host_inputs(inputs):
    """Per-core in_maps from the full problem inputs (layout/shard only)."""
    x = np.asarray(inputs["x"], dtype=np.float32)
    w_dw = np.asarray(inputs["w_dw"], dtype=np.float32)
    w_off = np.asarray(inputs["w_off"], dtype=np.float32)
    b_off = np.asarray(inputs["b_off"], dtype=np.float32)
    w_pc = np.asarray(inputs["w_pc"], dtype=np.float32)
    w_W = np.asarray(inputs["w_W"], dtype=np.float32)
    b_W = np.asarray(inputs["b_W"], dtype=np.float32)

    lin_w = np.linspace(-1.0, 1.0, W, dtype=np.float32)
    lin_h = np.linspace(-1.0, 1.0, H, dtype=np.float32)
    ident = np.eye(128, dtype=np.float32)
    zer = np.zeros((128, 128), dtype=np.float32)
    bw0 = np.zeros((128, C), dtype=np.float32)
    bw0[0, :] = b_W
    e0 = np.zeros((128, 128), dtype=np.float32)
    e0[0, :] = 1.0
    shared = {
        "w_pc": np.ascontiguousarray(w_pc),
        "w_W_T": np.ascontiguousarray(w_W.T),
        "w_dw_p": np.ascontiguousarray(w_dw.reshape(2, 128).T) * 16.0,
        "w_off_T": np.ascontiguousarray(w_off.T),
        "b_off_b": b_off.reshape(1, 2),
        "bw0": bw0,
        "e0": e0,
        "id0": np.concatenate([ident, zer], axis=1),
        "id1": np.concatenate([zer, ident], axis=1),
        "gxb": lin_w.reshape(W, 1),
        "gyb": lin_h.reshape(1, H),
        "sel": _sel_const(),
    }
    in_maps = []
    for b in range(B):
        m = dict(shared)
        m["x"] = np.ascontiguousarray(x[b].reshape(C, HW)).astype(NP_BF16)
        in_maps.append(m)
    return in_maps


def kernel_with_results(trace=False, **inputs):
    if "nc" not in _CACHE:
        _CACHE["nc"] = _build()
    nc = _CACHE["nc"]
    in_maps = _host_inputs(inputs)
    res = bass_utils.run_bass_kernel_spmd(
        nc, in_maps, core_ids=list(range(N_CORES)), trace=trace
    )
    outs = []
    for b in range(B):
        o = np.asarray(res.results[b]["out_pm"]).astype(np.float32)
        outs.append(o.reshape(H, W, C).transpose(2, 0, 1))
    return np.stack(outs, axis=0), res


def kernel(**inputs) -> np.ndarray:
    out, _ = kernel_with_results(**inputs)
    return out


# revision 39
# speedup vs baseline: 1.2957x; 1.2957x over previous
"""CDConvBlock Trainium2 kernel (8-core SPMD, data-parallel over batch).

Math (per image, x: [C, H*W] channel-major):
    offset = tanh(w_off @ x + b_off)                      [2, HW]
    gx = clip(gx_base + offset[0], -1, 1), gy likewise
    A' = (w_W @ w_pc) * w_dw[None, :]                     [C, C]
    out = bilinear_zeros(A' @ x, gx, gy) + b_W + x
The two 1x1 convs and the depthwise scale commute with the per-channel
bilinear gather, so they all fold into the single matrix A' applied
BEFORE sampling.

Device pipeline (per core = one image), software-pipelined so phase D
starts while phase B is still streaming:
  B-pass 1 (y path): stream x (bf16, host-cast).  Per 128-pixel chunk
     an x-stationary matmul against rhs = [A'.T | w_off.T] yields a
     PIXEL-major psum [128px, 258] = [y row | dx_pre | dy_pre]; y rows
     are cast to fp8 and written to DRAM twice in a PAIR layout (token
     (r, x) holds rows r AND r+1, 512 ch) so one gather element later
     covers all 4 bilinear corners.  Per 16-row batch, offset math on
     small [128, 16] tiles (pipelined one batch behind the matmuls)
     produces per-batch weight tiles wgt4_t[x, k, y] and gather-index
     tiles ibT_t (selector matmuls fold the pixel-major index field
     into the DMA-gather 16-partition-wrapped layout).
  B-pass 2 (residual path, drains on PE/ACT while phase D runs):
     re-stream x, identity matmuls transpose it to pixel-major, a
     one-hot matmul injects b_W, and the x+b_W seed is written
     straight to the output tensor.
  D (per 8-row chunk): ONE dma_gather fetches 1KB elements (2 adjacent
     pair tokens = 4 corners x 256 ch fp8); the gather source AP is
     windowed to rows <= 8k+72 so each chunk only waits on the y
     writes it can actually read.  DVE combine: one broadcast multiply
     M = G * wgt4[..., bcast c] and a 3-add tree t = (M0+M1)+(M2+M3);
     a gpsimd DMA accumulates t onto the pre-written seed
     (accum_op=add), two chunks per write.
Host only shards/reshapes (batch split, bf16 cast, weight transposes,
constant tables, final layout transpose).
"""

import numpy as np

import concourse.bass as bass
import concourse.bacc as bacc
import concourse.mybir as mybir
from concourse.bass_types import AP
from concourse.tile import TileContext
from concourse.tile_rust import add_dep_helper
from concourse import bass_utils
from concourse import library_config

F32 = mybir.dt.float32
BF16 = mybir.dt.bfloat16
I16 = mybir.dt.int16
FP8 = mybir.dt.float8e4
NP_BF16 = mybir.dt.np(BF16)
OP = mybir.AluOpType
AF = mybir.ActivationFunctionType

B, C, H, W = 8, 256, 128, 128
HW = H * W
N_CORES = 8

XB = 16     # image rows per x-stream batch
GC = 8      # image rows per gather chunk

_CACHE = {}


def _build(reps=1):
    nc = bacc.Bacc(
        "TRN2", target_bir_lowering=False, debug=False, num_devices=N_CORES
    )
    x_d = nc.dram_tensor("x", [C, HW], BF16, kind="ExternalInput")
    wpc_d = nc.dram_tensor("w_pc", [C, C], F32, kind="ExternalInput")
    wwt_d = nc.dram_tensor("w_W_T", [C, C], F32, kind="ExternalInput")
    wdw_d = nc.dram_tensor("w_dw_p", [128, 2], F32, kind="ExternalInput")
    wofft_d = nc.dram_tensor("w_off_T", [C, 2], F32, kind="ExternalInput")
    boff_d = nc.dram_tensor("b_off_b", [1, 2], F32, kind="ExternalInput")
    bw0_d = nc.dram_tensor("bw0", [128, C], F32, kind="ExternalInput")
    e0_d = nc.dram_tensor("e0", [128, 128], F32, kind="ExternalInput")
    gxb_d = nc.dram_tensor("gxb", [W, 1], F32, kind="ExternalInput")
    gyb_d = nc.dram_tensor("gyb", [1, H], F32, kind="ExternalInput")
    id0_d = nc.dram_tensor("id0", [128, C], F32, kind="ExternalInput")
    id1_d = nc.dram_tensor("id1", [128, C], F32, kind="ExternalInput")
    sel_d = nc.dram_tensor("sel", [128, 8 * 128], F32, kind="ExternalInput")
    out_d = nc.dram_tensor("out_pm", [HW, C], BF16, kind="ExternalOutput")

    with TileContext(nc) as tc:
        for _ in range(reps):
            _kernel_body(
                nc, tc, x_d, wpc_d, wwt_d, wdw_d, wofft_d, boff_d, bw0_d,
                e0_d, gxb_d, gyb_d, id0_d, id1_d, sel_d, out_d,
            )
    nc.finalize()
    return nc


def _kernel_body(
    nc, tc, x_d, wpc_d, wwt_d, wdw_d, wofft_d, boff_d, bw0_d,
    e0_d, gxb_d, gyb_d, id0_d, id1_d, sel_d, out_d,
):
    v = nc.vector
    g = nc.gpsimd
    s = nc.scalar

    with (
        tc.tile_pool(name="persist", bufs=1) as pp,
        tc.tile_pool(name="wload", bufs=1) as wp,
        tc.tile_pool(name="scr", bufs=2) as scr,
        tc.tile_pool(name="xs", bufs=2) as xp,
        tc.tile_pool(name="yst", bufs=2) as yp,
        tc.tile_pool(name="xst", bufs=2) as xsp,
        tc.tile_pool(name="gat", bufs=3) as gp,
        tc.tile_pool(name="mul", bufs=2) as mp,
        tc.tile_pool(name="fld", bufs=2) as fp_,
        tc.tile_pool(name="psA", bufs=4, space="PSUM") as psa,
        tc.tile_pool(name="dram", bufs=1, space="DRAM") as dp,
    ):
        # gpsimd ucode library for DMAGatherAnt
        lib_load = g.load_library(library_config.mlp)

        # ---------------- Phase A: weight prep ----------------
        wpc_sb = [wp.tile([128, C], F32, tag=f"wpc{m}", name=f"wpc{m}") for m in range(2)]
        wwt_sb = [wp.tile([128, C], F32, tag=f"wwt{m}", name=f"wwt{m}") for m in range(2)]
        wdw_sb = wp.tile([128, 2], F32, tag="wdw", name="wdw")
        wofft_sb = [wp.tile([128, 2], F32, tag=f"woft{m}", name=f"woft{m}") for m in range(2)]
        for m in range(2):
            nc.sync.dma_start(wpc_sb[m][:], wpc_d.ap()[m * 128:(m + 1) * 128, :])
            nc.sync.dma_start(wwt_sb[m][:], wwt_d.ap()[m * 128:(m + 1) * 128, :])
            nc.sync.dma_start(
                wofft_sb[m][:], wofft_d.ap()[m * 128:(m + 1) * 128, :]
            )
        nc.sync.dma_start(wdw_sb[:], wdw_d.ap())

        boff_sb = pp.tile([128, 2], F32, tag="boff", name="boff")
        gxb_sb = pp.tile([128, 1], F32, tag="gxb", name="gxb")
        gyb_sb = pp.tile([128, H], F32, tag="gyb", name="gyb")
        nc.sync.dma_start(boff_sb[:], boff_d.ap().to_broadcast((128, 2)))
        nc.sync.dma_start(gxb_sb[:], gxb_d.ap())
        nc.sync.dma_start(gyb_sb[:], gyb_d.ap().to_broadcast((128, H)))

        id_sb = [pp.tile([128, C], BF16, tag=f"id{m}", name=f"id{m}") for m in range(2)]
        g.dma_start(id_sb[0][:], id0_d.ap())  # f32 -> bf16 cast
        g.dma_start(id_sb[1][:], id1_d.ap())
        e0_sb = pp.tile([128, 128], BF16, tag="e0", name="e0")
        bw0_sb = pp.tile([128, C], BF16, tag="bw0", name="bw0")
        g.dma_start(e0_sb[:], e0_d.ap())
        g.dma_start(bw0_sb[:], bw0_d.ap())
        sel_sb = pp.tile([128, 8 * 128], F32, tag="sel", name="sel")
        nc.sync.dma_start(sel_sb[:], sel_d.ap())

        # B matrices: B[cb] = [A'.T | w_off.T] rows cb*128..cb*128+128
        B_sb = [pp.tile([128, C + 2], BF16, tag=f"B{cb}", name=f"B{cb}") for cb in range(2)]
        for cb in range(2):
            ps = psa.tile([128, 2, 512], F32, tag="psA", name="psA")
            for m in range(2):
                nc.tensor.matmul(
                    ps[:, 0, 0:C],
                    wpc_sb[m][:, cb * 128:(cb + 1) * 128],
                    wwt_sb[m][:],
                    start=(m == 0),
                    stop=(m == 1),
                )
            v.tensor_scalar(
                B_sb[cb][:, 0:C], ps[:, 0, 0:C], wdw_sb[:, cb:cb + 1], None,
                OP.mult,
            )
            v.tensor_copy(B_sb[cb][:, C:C + 2], wofft_sb[cb][:])

        # per-batch state tiles (small, one set per x-stream batch so
        # phase-D consumers depend only on their batch, not all of phase B)
        n_xbat = H // XB
        offs_t = [pp.tile([128, 2, XB], F32, tag=f"offs{t}", name=f"offs{t}")
                  for t in range(n_xbat)]
        wgt4_t = [pp.tile([128, 4, XB], BF16, tag=f"wgt4_{t}", name=f"wgt4_{t}")
                  for t in range(n_xbat)]
        ibT_t = [pp.tile([128, XB * 8], I16, tag=f"ibT{t}", name=f"ibT{t}")
                 for t in range(n_xbat)]

        # PAIR-token layout: token j = y*128+x holds fp8 y rows y and y+1
        # (512 ch).  8.4 MB.
        y_dram = dp.tile([HW * 2 * C], FP8, tag="ydram", name="ydram")

        # Pair row 127's bottom half (y row 128) is never gathered but sits
        # inside the gather source window — zero it so validation passes.
        zpad = pp.tile([128, C], FP8, tag="zpad", name="zpad")
        v.memset(zpad[:], 0.0)

        # ---------------- Phase C helpers (per batch) ----------------
        def axis_pipeline(o, base_is_tensor, t, eng=v):
            """offs_t[t][:, o, :] -> (b (f32 col base), wA, wB) for one axis."""
            ys, ye = t * XB, (t + 1) * XB
            d = scr.tile([128, XB], F32, tag="sc_d", name="sc_d")
            s.activation(d, offs_t[t][:, o, :], AF.Tanh, bias=boff_sb[:, o:o + 1])
            gg = scr.tile([128, XB], F32, tag="sc_g", name="sc_g")
            if base_is_tensor:
                eng.scalar_tensor_tensor(gg, d, 1.0, gyb_sb[:, ys:ye], OP.mult, OP.add)
            else:
                eng.tensor_scalar(gg, d, gxb_sb[:, 0:1], None, OP.add)
            # u = ix + 384 lives in the f32 binade [256, 512) where ulp is
            # 2^-15, so clearing the low 15 mantissa bits IS floor(u) for
            # the whole range ix in [-0.5, 127.5].  All exact.
            ix = scr.tile([128, XB], F32, tag="sc_ix", name="sc_ix")
            eng.tensor_scalar(ix, gg, -1.0, 1.0, OP.max, OP.min)
            eng.tensor_scalar(ix, ix, 64.0, 63.5 + 384.0, OP.mult, OP.add)
            ufl = scr.tile([128, XB], F32, tag="sc_ufl", name="sc_ufl")
            eng.tensor_scalar(
                ufl[:].bitcast(mybir.dt.uint32), ix[:].bitcast(mybir.dt.uint32),
                0xFFFF8000, None, OP.bitwise_and,
            )
            fx = scr.tile([128, XB], F32, tag="sc_fx", name="sc_fx")
            eng.tensor_tensor(fx, ix, ufl, OP.subtract)
            mneg = scr.tile([128, XB], F32, tag="sc_mneg", name="sc_mneg")
            eng.tensor_scalar(mneg, ufl, 383.5, None, OP.is_lt)
            mhi = scr.tile([128, XB], F32, tag="sc_mhi", name="sc_mhi")
            eng.tensor_scalar(mhi, ufl, 510.5, None, OP.is_gt)
            # clamped base, still in u-domain (bx + 384)
            bcol = scr.tile([128, XB], F32, tag="sc_b", name="sc_b")
            eng.tensor_scalar(bcol, ufl, 384.0, 510.0, OP.max, OP.min)
            # common = 1 - mneg - mhi;  g1 = 1 - fx
            cm = scr.tile([128, XB], F32, tag="sc_cm", name="sc_cm")
            eng.tensor_tensor(cm, mneg, mhi, OP.add)
            eng.tensor_scalar(cm, cm, -1.0, 1.0, OP.mult, OP.add)
            g1 = scr.tile([128, XB], F32, tag="sc_g1", name="sc_g1")
            eng.tensor_scalar(g1, fx, -1.0, 1.0, OP.mult, OP.add)
            # wA = g1*common + fx*mneg ; wB = fx*common + g1*mhi
            wA = scr.tile([128, XB], F32, tag="sc_wA", name="sc_wA")
            t1 = scr.tile([128, XB], F32, tag="sc_t1", name="sc_t1")
            eng.tensor_tensor(wA, g1, cm, OP.mult)
            eng.tensor_tensor(t1, fx, mneg, OP.mult)
            eng.tensor_tensor(wA, wA, t1, OP.add)
            eng.tensor_scalar(wA, wA, 0.25, None, OP.mult)
            wB_ = scr.tile([128, XB], F32, tag="sc_wB", name="sc_wB")
            eng.tensor_tensor(wB_, fx, cm, OP.mult)
            eng.tensor_tensor(t1, g1, mhi, OP.mult)
            eng.tensor_tensor(wB_, wB_, t1, OP.add)
            eng.tensor_scalar(wB_, wB_, 0.25, None, OP.mult)
            return bcol, wA, wB_

        idf_t = {}

        def build_weights(t):
            """DVE/ACT part: corner weights + f32 token index for batch t."""
            bx, wL, wR = axis_pipeline(0, False, t)
            by, wT, wB_ = axis_pipeline(1, True, t)
            # gather corner order within a 1KB element: [v00, v10, v01, v11]
            v.tensor_tensor(wgt4_t[t][:, 0, :], wT, wL, OP.mult)
            v.tensor_tensor(wgt4_t[t][:, 1, :], wB_, wL, OP.mult)
            v.tensor_tensor(wgt4_t[t][:, 2, :], wT, wR, OP.mult)
            v.tensor_tensor(wgt4_t[t][:, 3, :], wB_, wR, OP.mult)
            # pair-token index: idx = by*128 + bx.  bx/by are still in the
            # u-domain (+384 each): subtract 384*129.
            idf = pp.tile([128, XB], F32, tag=f"idf{t}", name=f"idf{t}")
            v.scalar_tensor_tensor(idf, by, 128.0, bx, OP.mult, OP.add)
            v.tensor_scalar(idf, idf, -384.0 * 129.0, None, OP.add)
            idf_t[t] = idf

        def build_indices(t):
            """PE part: fold idf into the gather index layout
            ibT_t[t][p, y*8+g] = idf[g*16 + p%16, y] via 8 selector matmuls
            into disjoint psum columns + one strided cast copy."""
            psS = psa.tile([128, 2, 512], F32, tag="psA", name="psS")
            for q in range(8):
                nc.tensor.matmul(
                    psS[:, 0, q * XB:(q + 1) * XB],
                    sel_sb[:, q * 128:(q + 1) * 128],
                    idf_t[t][:],
                    start=True,
                    stop=True,
                    skip_group_check=True,
                )
            src = (
                psS[:, 0, 0:XB * 8]
                .rearrange("p (q y) -> p q y", y=XB)
                .transpose([0, 2, 1])
            )
            v.tensor_copy(
                ibT_t[t][:].rearrange("p (y q) -> p y q", q=8), src
            )

        # ---------------- Phase B: matmuls, y tokens, residual seed ----------------
        y_writes = []       # pair-token writes (gather RAW deps)
        seed_writes = []    # out_pm seed writes (accum RAW deps)
        y2full = y_dram[:].rearrange("(j c) -> j c", c=2 * C)
        zv = y2full[(H - 1) * 128:HW, C:2 * C].rearrange("(r x) c -> x r c", x=128)
        zpad_write = nc.sync.dma_start(zv, zpad[:].unsqueeze(1))
        # ---- pass 1: y-token path only, so gathers can start early ----
        for t in range(n_xbat):
            x0 = xp.tile([128, XB * 128], BF16, tag="x0", name="x0")
            x1 = xp.tile([128, XB * 128], BF16, tag="x1", name="x1")
            cols = slice(t * XB * 128, (t + 1) * XB * 128)
            nc.sync.dma_start(x0[:], x_d.ap()[0:128, cols])
            nc.sync.dma_start(x1[:], x_d.ap()[128:256, cols])
            # weights/index math for the PREVIOUS batch: its ACT/DVE ops run
            # while this batch's matmuls stream, and the PE fold lands after
            # them, so no engine waits on a cross-engine chain.
            if t > 0:
                build_weights(t - 1)
            yst = yp.tile([128, XB, C], FP8, tag="yst", name="yst")
            for hb in range(XB // 2):
                psA = psa.tile([128, 2, 512], F32, tag="psA", name="psA")
                for r in range(2):
                    cc = (hb * 2 + r) * 128
                    lhs0 = x0[:, cc:cc + 128]
                    lhs1 = x1[:, cc:cc + 128]
                    nc.tensor.matmul(
                        psA[:, r, 0:C + 2], lhs0, B_sb[0][:], start=True, stop=False
                    )
                    nc.tensor.matmul(
                        psA[:, r, 0:C + 2], lhs1, B_sb[1][:], start=False, stop=True
                    )
                rr = hb * 2
                # y tokens -> fp8 staging (ACT)
                s.copy(yst[:, rr:rr + 2, :], psA[:, :, 0:C])
                # offsets: [x, r, o] -> offs_t[t][x, o, rr+r]
                v.tensor_copy(
                    offs_t[t][:, :, rr:rr + 2].transpose([0, 2, 1]),
                    psA[:, :, C:C + 2],
                )
            if t > 0:
                build_indices(t - 1)
            rows = slice(t * XB * 128, (t + 1) * XB * 128)
            # write 1: top half of pair tokens (t*XB .. t*XB+XB-1)
            yv1 = y2full[rows, 0:C].rearrange("(r x) c -> x r c", x=128)
            y_writes.append(nc.sync.dma_start(yv1, yst[:]))
            # write 2: bottom half of pair tokens (t*XB-1 .. t*XB+XB-2)
            if t == 0:
                yv2 = y2full[0:(XB - 1) * 128, C:2 * C].rearrange(
                    "(r x) c -> x r c", x=128
                )
                y_writes.append(nc.sync.dma_start(yv2, yst[:, 1:XB, :]))
            else:
                yv2 = y2full[(t * XB - 1) * 128:(t * XB - 1 + XB) * 128, C:2 * C].rearrange(
                    "(r x) c -> x r c", x=128
                )
                y_writes.append(nc.sync.dma_start(yv2, yst[:]))
        build_weights(n_xbat - 1)
        build_indices(n_xbat - 1)

        # ---- pass 2: residual transpose path; PE/ACT drain it during
        # phase D (x is re-streamed, DMA has headroom there) ----
        for t in range(n_xbat):
            x0 = xp.tile([128, XB * 128], BF16, tag="x0", name="x0")
            x1 = xp.tile([128, XB * 128], BF16, tag="x1", name="x1")
            cols = slice(t * XB * 128, (t + 1) * XB * 128)
            nc.sync.dma_start(x0[:], x_d.ap()[0:128, cols])
            nc.sync.dma_start(x1[:], x_d.ap()[128:256, cols])
            xst = xsp.tile([128, XB, C], BF16, tag="xst", name="xst")
            for hb in range(XB // 2):
                psB = psa.tile([128, 2, 512], F32, tag="psA", name="psB")
                for r in range(2):
                    cc = (hb * 2 + r) * 128
                    nc.tensor.matmul(
                        psB[:, r, 0:C], x0[:, cc:cc + 128], id_sb[0][:],
                        start=True, stop=False,
                    )
                    nc.tensor.matmul(
                        psB[:, r, 0:C], x1[:, cc:cc + 128], id_sb[1][:],
                        start=False, stop=False,
                    )
                    # inject b_W: psB[m, :] += bw0[0, :]  (E0 one-hot)
                    nc.tensor.matmul(
                        psB[:, r, 0:C], e0_sb[:], bw0_sb[:], start=False, stop=True
                    )
                rr = hb * 2
                # residual seed x + b_W -> bf16 staging (ACT)
                s.copy(xst[:, rr:rr + 2, :], psB[:, :, 0:C])
            rows = slice(t * XB * 128, (t + 1) * XB * 128)
            ov = out_d.ap()[rows, :].rearrange("(r x) c -> x r c", x=128)
            seed_writes.append(nc.sync.dma_start(ov, xst[:]))

        # ---------------- Phase D: gather + bilinear combine ----------------
        n_chunk = H // GC
        nidx = GC * 128
        for k in range(n_chunk):
            gD = gp.tile([128, GC, 4 * C], FP8, tag="gD", name="gD")
            tb = k // 2
            half = (k % 2) * (XB // 2) * 8
            bmax = min(n_xbat - 1, (GC * k + GC - 1 + 65) // XB)
            # gather source window: pair rows < wrows (limits RAW deps).
            # Exact bound: chunk k touches pair rows <= 8k+71, whose bottom
            # halves come from y rows <= 8k+72.
            wrows = min(H, GC * k + GC + 65)
            y2d = AP(
                y_dram[:].tensor, y_dram[:].offset,
                [[2 * C, wrows * 128 - 1], [1, 4 * C]],
            )
            gi = g.dma_gather(
                gD[:], y2d, ibT_t[tb][:, half:half + nidx // 16], nidx, nidx,
                elem_size=4 * C, elem_step=2 * C,
            )
            add_dep_helper(gi.ins, lib_load.ins, reason="gather needs mlp lib")
            if wrows >= H:
                add_dep_helper(gi.ins, zpad_write.ins, reason="gather window pad")
            for wr in y_writes[:2 * (bmax + 1)]:
                add_dep_helper(gi.ins, wr.ins, reason="gather reads y_dram")
            # combine: M = G * w  (weight bcast over channels), then the
            # 3-add fold tree; the out DMA accumulates onto the seed.
            M = mp.tile([128, GC, 4, C], BF16, tag="M", name="M")
            wv = (
                wgt4_t[tb][:, :, (k % 2) * GC:(k % 2) * GC + GC]
                .transpose([0, 2, 1])
                .unsqueeze(3)
                .broadcast_to((128, GC, 4, C))
            )
            gv = gD[:].rearrange("p y (k c) -> p y k c", k=4)
            v.tensor_tensor(M[:], gv, wv, OP.mult)
            A_ = fp_.tile([128, GC, C], BF16, tag="fa", name="fa")
            B2 = fp_.tile([128, GC, C], BF16, tag="fb", name="fb")
            if k % 2 == 0:
                T2 = fp_.tile([128, 2, GC, C], BF16, tag="ft", name="ft")
            v.tensor_tensor(A_[:], M[:, :, 0, :], M[:, :, 1, :], OP.add)
            v.tensor_tensor(B2[:], M[:, :, 2, :], M[:, :, 3, :], OP.add)
            v.tensor_tensor(T2[:, k % 2], A_[:], B2[:], OP.add)
            if k % 2 == 1:
                ov = out_d.ap()[(k - 1) * nidx:(k + 1) * nidx, :].rearrange(
                    "(r x) c -> x r c", x=128
                )
                ow = g.dma_start(ov, T2[:], accum_op=OP.add)
                add_dep_helper(
                    ow.ins, seed_writes[(GC * k) // XB].ins,
                    reason="accum needs seed written first",
                )


def _sel_const():
    sel = np.zeros((128, 8, 128), dtype=np.float32)
    for gq in range(8):
        for p in range(128):
            sel[gq * 16 + p % 16, gq, p] = 1.0
    return sel.reshape(128, 8 * 128)


def _host_inputs(inputs):
    """Per-core in_maps from the full problem inputs (layout/shard only)."""
    x = np.asarray(inputs["x"], dtype=np.float32)
    w_dw = np.asarray(inputs["w_dw"], dtype=np.float32)
    w_off = np.asarray(inputs["w_off"], dtype=np.float32)
    b_off = np.asarray(inputs["b_off"], dtype=np.float32)
    w_pc = np.asarray(inputs["w_pc"], dtype=np.float32)
    w_W = np.asarray(inputs["w_W"], dtype=np.float32)
    b_W = np.asarray(inputs["b_W"], dtype=np.float32)

    lin_w = np.linspace(-1.0, 1.0, W, dtype=np.float32)
    lin_h = np.linspace(-1.0, 1.0, H, dtype=np.float32)
    ident = np.eye(128, dtype=np.float32)
    zer = np.zeros((128, 128), dtype=np.float32)
    bw0 = np.zeros((128, C), dtype=np.float32)
    bw0[0, :] = b_W
    e0 = np.zeros((128, 128), dtype=np.float32)
    e0[0, :] = 1.0
    shared = {
        "w_pc": np.ascontiguousarray(w_pc),
        "w_W_T": np.ascontiguousarray(w_W.T),
        "w_dw_p": np.ascontiguousarray(w_dw.reshape(2, 128).T) * 16.0,
        "w_off_T": np.ascontiguousarray(w_off.T),
        "b_off_b": b_off.reshape(1, 2),
        "bw0": bw0,
        "e0": e0,
        "id0": np.concatenate([ident, zer], axis=1),
        "id1": np.concatenate([zer, ident], axis=1),
        "gxb": lin_w.reshape(W, 1),
        "gyb": lin_h.reshape(1, H),
        "sel": _sel_const(),
    }
    in_maps = []
    for b in range(B):
        m = dict(shared)
        m["x"] = np.ascontiguousarray(x[b].reshape(C, HW)).astype(NP_BF16)
        in_maps.append(m)
    return in_maps


def kernel_with_results(trace=False, **inputs):
    if "nc" not in _CACHE:
        _CACHE["nc"] = _build()
    nc = _CACHE["nc"]
    in_maps = _host_inputs(inputs)
    res = bass_utils.run_bass_kernel_spmd(
        nc, in_maps, core_ids=list(range(N_CORES)), trace=trace
    )
    outs = []
    for b in range(B):
        o = np.asarray(res.results[b]["out_pm"]).astype(np.float32)
        outs.append(o.reshape(H, W, C).transpose(2, 0, 1))
    return np.stack(outs, axis=0), res


def kernel(**inputs) -> np.ndarray:
    out, _ = kernel_with_results(**inputs)
    return out
